# revision 6
# baseline (speedup 1.0000x reference)
"""Trainium2 Bass kernel for nn_MultiHeadAttention (B=2, S=4096, D=512, H=8).

Computes: q/k/v = relu(x@W+b) per head, softmax(q k^T / sqrt(64)) v,
out = relu(concat_heads @ Wo + bo).

Sharding: 8 cores = 2 (batch) x 4 (query-slice).  Each core computes full
K/V projections for its batch (redundant across the 4 q-slice cores) and
attention + output projection for its 1024-row query slice.  No collectives;
the host concatenates the 8 output slices.

Host-side prep (part of the sharding/layout step, not device compute):
x is cast to bf16 and transposed to feature-major x^T per batch, and the
weight matrices are cast to bf16 — the tensor engine contracts along the
partition dim, so all device matmuls consume feature-major operands.

Per-core kernel (all matmuls bf16 with fp32 PSUM accumulation):
  - K^T, Q^T computed feature-major: lhsT=W tile, rhs=x^T.  Bias+relu fused
    on DVE (bias is per-partition in this layout).
  - V computed in natural [s, d] layout (lhsT = x^T tile, rhs = Wv); bias via
    a K=1 ones-row matmul; relu on DVE; stored per head with a ones column
    appended (V_pad) so the attention U matmul also produces the softmax
    denominator row for free.
  - scores^T = K^T_h.T @ Q^T_h per (head, ktile): K=64 contraction; heads are
    processed in pairs at base partitions 0/64 so the two matmuls run
    concurrently in different PE row-groups.
  - exp on ACT (scale=1/8 fused), no max-subtraction (relu'd q/k make scores
    bounded: measured range [0, 6.6]).  ACT exp is the kernel's throughput
    floor (~1 elem/lane/cycle): exp ops span 2 ktiles x 2 heads (4 PSUM
    banks) to amortize the per-op overhead, the first attention block is
    interleaved with the K/V projection chunks, and the remaining
    projections are emitted between attention blocks so the PE does them
    inside ACT-bound stretches.
  - U^T[65, q] = V_pad_h.T @ P^T accumulated over ktiles in PSUM; row 64 is
    the denominator.  U^T is copied to SBUF immediately (releases the PSUM
    accumulator for the next block), then normalized off the critical path:
    DVE reciprocal + gpsimd partition broadcast + DVE multiply into
    feature-major O^T.
  - out = relu(O^T.T @ Wo + bo) via lhsT=O^T tiles, rhs=Wo; bias via ones-row
    matmul; relu on ACT; DMA to HBM.
"""

import numpy as np
import ml_dtypes

import concourse.bass as bass
import concourse.mybir as mybir
import concourse.tile as tile
from concourse import bacc
from concourse import bass_utils

F32 = mybir.dt.float32
BF16 = mybir.dt.bfloat16
AF = mybir.ActivationFunctionType
ALU = mybir.AluOpType

P = 128
D = 512
H = 8
DH = 64
DT = D // P  # 4 (also = number of head pairs)
B = 2
S = 4096
NCORES = 8
QSPLIT = 4
SQ_FULL = S // QSPLIT  # 1024 query rows per core
QC = 512               # q-chunk (matmul free dim / PSUM bank width)


def build_mha(sk=S, sq=SQ_FULL):
    """Build the SPMD Bass program (identical on all cores)."""
    nc = bacc.Bacc("TRN2", target_bir_lowering=False, debug=False,
                   num_devices=NCORES)

    xT_d = nc.dram_tensor("xT_bf", (D, sk), BF16, kind="ExternalInput").ap()
    xqT_d = nc.dram_tensor("xqT_bf", (D, sq), BF16, kind="ExternalInput").ap()
    w_dram = {}
    for n in ("wq", "wk", "wv", "wo"):
        w_dram[n] = nc.dram_tensor(n, (D, D), BF16, kind="ExternalInput").ap()
    b_dram = {
        "bq": nc.dram_tensor("bq", (D,), F32, kind="ExternalInput").ap(),
        "bk": nc.dram_tensor("bk", (D,), F32, kind="ExternalInput").ap(),
        "bv": nc.dram_tensor("bv", (D,), BF16, kind="ExternalInput").ap(),
        "bo": nc.dram_tensor("bo", (D,), BF16, kind="ExternalInput").ap(),
    }
    out = nc.dram_tensor("out", (sq, D), F32, kind="ExternalOutput").ap()

    with tile.TileContext(nc) as tc:
        _build_tile(tc, xT_d, xqT_d, w_dram, b_dram, out, sk, sq)

    nc.compile()
    return nc


def _build_tile(tc, xT_d, xqT_d, w_dram, b_dram, out, sk, sq):
    nc = tc.nc
    SK_T = sk // P            # ktiles of the key/value sequence
    SQ_T = sq // P
    NQC = sq // QC            # q chunks per core
    CH = min(4, SK_T)         # stiles per projection chunk
    NCH = SK_T // CH
    KG = 2                    # ktiles per exp group

    with (
        tc.tile_pool(name="singles", bufs=1) as singles,
        tc.tile_pool(name="work", bufs=3) as work,
        tc.tile_pool(name="psum", bufs=2, space="PSUM") as psum,
    ):
        xT_src = xT_d.rearrange("(t p) s -> p t s", p=P)

        # ---- startup: only what Q-proj pair 0 needs, first ----
        w_bf = {}
        w_bf["wq"] = singles.tile([P, DT, D], BF16, name="wq_bf")
        nc.sync.dma_start(w_bf["wq"], w_dram["wq"].rearrange(
            "(t p) n -> p t n", p=P))
        b_col = {}
        b_col["bq"] = singles.tile([P, DT], F32, name="bq_col")
        nc.sync.dma_start(b_col["bq"], b_dram["bq"].rearrange(
            "(t p) -> p t", p=P))
        xTq = singles.tile([P, DT, sq], BF16)
        nc.sync.dma_start(xTq, xqT_d.rearrange("(t p) s -> p t s", p=P))

        QT = singles.tile([P, DT, sq], BF16)

        def qproj(j, nq):
            psQ = psum.tile([P, QC], F32, tag="proj", name="psQ")
            for kt in range(DT):
                nc.tensor.matmul(
                    psQ, w_bf["wq"][:, kt, j * P:(j + 1) * P],
                    xTq[:, kt, nq * QC:(nq + 1) * QC],
                    start=(kt == 0), stop=(kt == DT - 1))
            nc.vector.tensor_scalar(
                QT[:, j, nq * QC:(nq + 1) * QC], psQ,
                b_col["bq"][:, j:j + 1], 0.0, op0=ALU.add, op1=ALU.max)

        qproj(0, 0)

        # ---- rest of the weights / biases ----
        for n in ("wk", "wv", "wo"):
            wb = singles.tile([P, DT, D], BF16, name=f"{n}_bf")
            nc.sync.dma_start(wb, w_dram[n].rearrange("(t p) n -> p t n", p=P))
            w_bf[n] = wb
        b_col["bk"] = singles.tile([P, DT], F32, name="bk_col")
        nc.sync.dma_start(b_col["bk"], b_dram["bk"].rearrange(
            "(t p) -> p t", p=P))
        b_row = {}
        for n in ("bv", "bo"):
            br = singles.tile([1, D], BF16, name=f"{n}_row")
            nc.sync.dma_start(br, b_dram[n][None, :])
            b_row[n] = br

        # ---- persistent SBUF tensors ----
        xT = singles.tile([P, DT, sk], BF16)
        xT1 = singles.tile([1, sk], BF16)
        nc.vector.memset(xT1, 1.0)
        KT = singles.tile([P, DT, sk], BF16)
        V_pad = singles.tile([P, SK_T, H, DH + 1], BF16)
        nc.vector.memset(V_pad[:, :, :, DH:DH + 1], 1.0)
        OT = singles.tile([P, DT, sq], BF16)
        OT1 = singles.tile([1, sq], BF16)
        nc.vector.memset(OT1, 1.0)

        # PSUM tags: "proj" 2x1 banks, "scores" 1x4 banks, "psU" 2x1 = 8
        def vproj(st):
            psV = psum.tile([P, D], F32, tag="proj", name="psV")
            for kt in range(DT):
                nc.tensor.matmul(
                    psV, xT[:, kt, st * P:(st + 1) * P], w_bf["wv"][:, kt, :],
                    start=(kt == 0), stop=False)
            nc.tensor.matmul(psV, xT1[:, st * P:(st + 1) * P],
                             b_row["bv"], start=False, stop=True)
            nc.vector.tensor_scalar_max(
                V_pad[:, st, :, 0:DH],
                psV.rearrange("p (h d) -> p h d", h=H), 0.0)

        def kproj(j, n):
            psK = psum.tile([P, CH * P], F32, tag="proj", name="psK")
            for kt in range(DT):
                nc.tensor.matmul(
                    psK, w_bf["wk"][:, kt, j * P:(j + 1) * P],
                    xT[:, kt, n * CH * P:(n + 1) * CH * P],
                    start=(kt == 0), stop=(kt == DT - 1))
            nc.vector.tensor_scalar(
                KT[:, j, n * CH * P:(n + 1) * CH * P], psK,
                b_col["bk"][:, j:j + 1], 0.0, op0=ALU.add, op1=ALU.max)

        def attn_group(j, qc, kt0, nkt, psU_A, psU_B):
            """One exp group: nkt ktiles x 2 heads -> one ACT exp op."""
            q0 = qc * QC
            psS = psum.tile([P, 2 * KG * QC], F32, tag="scores", bufs=1,
                            name="psS")
            for i in range(nkt):
                kt = kt0 + i
                nc.tensor.matmul(
                    psS[:, (2 * i) * QC:(2 * i + 1) * QC],
                    KT[0:DH, j, kt * P:(kt + 1) * P],
                    QT[0:DH, j, q0:q0 + QC], start=True, stop=True)
                nc.tensor.matmul(
                    psS[:, (2 * i + 1) * QC:(2 * i + 2) * QC],
                    KT[DH:P, j, kt * P:(kt + 1) * P],
                    QT[DH:P, j, q0:q0 + QC], start=True, stop=True)
            pT = work.tile([P, 2 * KG * QC], BF16, tag="pT", name="pT")
            nc.scalar.activation(pT[:, :2 * nkt * QC], psS[:, :2 * nkt * QC],
                                 AF.Exp, scale=0.125)
            for i in range(nkt):
                kt = kt0 + i
                first, last = (kt == 0), (kt == SK_T - 1)
                nc.tensor.matmul(psU_A, V_pad[:, kt, 2 * j, :],
                                 pT[:, (2 * i) * QC:(2 * i + 1) * QC],
                                 start=first, stop=last)
                nc.tensor.matmul(psU_B, V_pad[:, kt, 2 * j + 1, :],
                                 pT[:, (2 * i + 1) * QC:(2 * i + 2) * QC],
                                 start=first, stop=last)

        def attn_finish(j, qc, psU_A, psU_B):
            """Copy U out of PSUM fast (frees both accumulators), then
            normalize off the critical path."""
            q0 = qc * QC
            ucs = []
            for psU in (psU_A, psU_B):
                uc = work.tile([DH + 1, QC], F32, tag="ucopy", bufs=4,
                               name="uc")
                nc.vector.tensor_copy(uc, psU)
                ucs.append(uc)
            for uc, h0 in zip(ucs, (0, DH)):
                recip = work.tile([1, QC], F32, tag="recip", bufs=4,
                                  name="recip")
                nc.vector.reciprocal(recip, uc[DH:DH + 1, :])
                brc = work.tile([DH, QC], F32, tag="brc", bufs=4, name="brc")
                nc.gpsimd.partition_broadcast(brc, recip)
                nc.vector.tensor_mul(
                    OT[h0:h0 + DH, j, q0:q0 + QC], uc[0:DH, :], brc)

        def attn_span(j, qc, kts, psU):
            for kt0 in range(kts[0], kts[0] + len(kts), KG):
                nkt = min(KG, kts[-1] + 1 - kt0)
                attn_group(j, qc, kt0, nkt, psU[0], psU[1])
            if kts[-1] == SK_T - 1:
                attn_finish(j, qc, psU[0], psU[1])

        def new_psU():
            a = psum.tile([DH + 1, QC], F32, tag="psU", name="psU_A")
            b = psum.tile([DH + 1, QC], F32, tag="psU", name="psU_B")
            return (a, b)

        # ---- chunk loop: x load + V proj + K proj(pair 0) + attn(0, 0) ----
        psU0 = new_psU()
        for n in range(NCH):
            s0, s1 = n * CH * P, (n + 1) * CH * P
            nc.sync.dma_start(xT[:, :, s0:s1], xT_src[:, :, s0:s1])
            for st in range(n * CH, (n + 1) * CH):
                vproj(st)
            kproj(0, n)
            attn_span(0, 0, list(range(n * CH, (n + 1) * CH)), psU0)

        # ---- remaining attention; projections hidden between blocks ----
        for qc in range(1, NQC):
            qproj(0, qc)
            psU01 = new_psU()
            attn_span(0, qc, list(range(SK_T)), psU01)
        for j in range(1, DT):
            for n in range(NCH):
                kproj(j, n)
            for qc in range(NQC):
                qproj(j, qc)
                psU = new_psU()
                attn_span(j, qc, list(range(SK_T)), psU)

        # ---- output projection ----
        for qt in range(SQ_T):
            psO = psum.tile([P, D], F32, tag="proj", name="psO")
            for j in range(DT):
                nc.tensor.matmul(psO, OT[:, j, qt * P:(qt + 1) * P],
                                 w_bf["wo"][:, j, :],
                                 start=(j == 0), stop=False)
            nc.tensor.matmul(psO, OT1[:, qt * P:(qt + 1) * P],
                             b_row["bo"], start=False, stop=True)
            o_sb = work.tile([P, D], F32, tag="osb", bufs=2, name="o_sb")
            nc.scalar.activation(o_sb, psO, AF.Relu)
            nc.sync.dma_start(out[qt * P:(qt + 1) * P, :], o_sb)


_NC_CACHE = {}


def _get_nc(sk=S, sq=SQ_FULL):
    key = (sk, sq)
    if key not in _NC_CACHE:
        _NC_CACHE[key] = build_mha(sk, sq)
    return _NC_CACHE[key]


def kernel(x, Wq, bq, Wk, bk, Wv, bv, Wo, bo, **run_kwargs):
    """Full-input entry point: shards across 8 NeuronCores, returns full out."""
    bf = ml_dtypes.bfloat16
    x = np.asarray(x, dtype=np.float32)
    w_bf = {
        "wq": np.ascontiguousarray(np.asarray(Wq, np.float32).astype(bf)),
        "wk": np.ascontiguousarray(np.asarray(Wk, np.float32).astype(bf)),
        "wv": np.ascontiguousarray(np.asarray(Wv, np.float32).astype(bf)),
        "wo": np.ascontiguousarray(np.asarray(Wo, np.float32).astype(bf)),
    }
    biases = {
        "bq": np.ascontiguousarray(np.asarray(bq, np.float32)),
        "bk": np.ascontiguousarray(np.asarray(bk, np.float32)),
        "bv": np.ascontiguousarray(np.asarray(bv, np.float32).astype(bf)),
        "bo": np.ascontiguousarray(np.asarray(bo, np.float32).astype(bf)),
    }
    # host-side layout prep: bf16 cast + feature-major transpose per batch
    xT_b = [np.ascontiguousarray(x[b].T.astype(bf)) for b in range(B)]

    nc = _get_nc()
    in_maps = []
    for c in range(NCORES):
        b, qo = divmod(c, QSPLIT)
        m = dict(w_bf)
        m.update(biases)
        m["xT_bf"] = xT_b[b]
        m["xqT_bf"] = np.ascontiguousarray(
            xT_b[b][:, qo * SQ_FULL:(qo + 1) * SQ_FULL])
        in_maps.append(m)
    res = bass_utils.run_bass_kernel_spmd(
        nc, in_maps, core_ids=list(range(NCORES)), **run_kwargs)
    full = np.empty((B, S, D), np.float32)
    for c in range(NCORES):
        b, qo = divmod(c, QSPLIT)
        full[b, qo * SQ_FULL:(qo + 1) * SQ_FULL] = res.results[c]["out"]
    if run_kwargs:
        return full, res
    return full


# revision 8
# speedup vs baseline: 1.5868x; 1.5868x over previous
"""Trainium2 Bass kernel for nn_MultiHeadAttention (B=2, S=4096, D=512, H=8).

Computes: q/k/v = relu(x@W+b) per head, softmax(q k^T / sqrt(64)) v,
out = relu(concat_heads @ Wo + bo).

Sharding: 8 cores = 2 (batch) x 4 (query-slice).  Each core computes full
K/V projections for its batch (redundant across the 4 q-slice cores) and
attention + output projection for its 1024-row query slice.  No collectives;
the host concatenates the 8 output slices.

Host-side prep (part of the sharding/layout step, not device compute):
x is cast to bf16 and transposed to feature-major x^T per batch, and the
weight matrices are cast to bf16 — the tensor engine contracts along the
partition dim, so all device matmuls consume feature-major operands.

Per-core kernel (all matmuls bf16 with fp32 PSUM accumulation):
  - K^T, Q^T computed feature-major: lhsT=W tile, rhs=x^T.  Bias+relu fused
    on DVE (bias is per-partition in this layout).
  - V computed in natural [s, d] layout (lhsT = x^T tile, rhs = Wv); bias via
    a K=1 ones-row matmul; relu on DVE; stored per head with a ones column
    appended (V_pad) so the attention U matmul also produces the softmax
    denominator row for free.
  - scores^T = K^T_h.T @ Q^T_h per (head, ktile): K=64 contraction; heads are
    processed in pairs at base partitions 0/64 so the two matmuls run
    concurrently in different PE row-groups.
  - exp on ACT (scale=1/8 fused), no max-subtraction (relu'd q/k make scores
    bounded: measured range [0, 6.6]).  ACT exp is the kernel's throughput
    floor (~1 elem/lane/cycle): exp ops span 2 ktiles x 2 heads (4 PSUM
    banks) to amortize the per-op overhead, the first attention block is
    interleaved with the K/V projection chunks, and the remaining
    projections are emitted between attention blocks so the PE does them
    inside ACT-bound stretches.
  - U^T[65, q] = V_pad_h.T @ P^T accumulated over ktiles in PSUM; row 64 is
    the denominator.  U^T is copied to SBUF immediately (releases the PSUM
    accumulator for the next block), then normalized off the critical path:
    DVE reciprocal + gpsimd partition broadcast + DVE multiply into
    feature-major O^T.
  - out = relu(O^T.T @ Wo + bo) via lhsT=O^T tiles, rhs=Wo; bias via ones-row
    matmul; relu on ACT; DMA to HBM.
"""

import numpy as np
import ml_dtypes

import concourse.bass as bass
import concourse.mybir as mybir
import concourse.tile as tile
from concourse import bacc
from concourse import bass_utils

F32 = mybir.dt.float32
BF16 = mybir.dt.bfloat16
AF = mybir.ActivationFunctionType
ALU = mybir.AluOpType

P = 128
D = 512
H = 8
DH = 64
DT = D // P  # 4 (also = number of head pairs)
B = 2
S = 4096
NCORES = 8
QSPLIT = 4
SQ_FULL = S // QSPLIT  # 1024 query rows per core
QC = 512               # q-chunk (matmul free dim / PSUM bank width)


def build_mha(sk=S, sq=SQ_FULL):
    """Build the SPMD Bass program (identical on all cores)."""
    nc = bacc.Bacc("TRN2", target_bir_lowering=False, debug=False,
                   num_devices=NCORES)

    xT_d = nc.dram_tensor("xT_bf", (D, sk), BF16, kind="ExternalInput").ap()
    xqT_d = nc.dram_tensor("xqT_bf", (D, sq), BF16, kind="ExternalInput").ap()
    w_dram = {}
    for n in ("wq", "wk", "wv", "wo"):
        w_dram[n] = nc.dram_tensor(n, (D, D), BF16, kind="ExternalInput").ap()
    b_dram = {
        "bq": nc.dram_tensor("bq", (D,), F32, kind="ExternalInput").ap(),
        "bk": nc.dram_tensor("bk", (D,), F32, kind="ExternalInput").ap(),
        "bv": nc.dram_tensor("bv", (D,), BF16, kind="ExternalInput").ap(),
        "bo": nc.dram_tensor("bo", (D,), BF16, kind="ExternalInput").ap(),
    }
    out = nc.dram_tensor("out", (sq, D), F32, kind="ExternalOutput").ap()

    with tile.TileContext(nc) as tc:
        _build_tile(tc, xT_d, xqT_d, w_dram, b_dram, out, sk, sq)

    nc.compile()
    return nc


def _build_tile(tc, xT_d, xqT_d, w_dram, b_dram, out, sk, sq):
    nc = tc.nc
    SK_T = sk // P            # ktiles of the key/value sequence
    SQ_T = sq // P
    NQC = sq // QC            # q chunks per core
    CH = min(4, SK_T)         # stiles per projection chunk
    NCH = SK_T // CH
    KG = 1                    # ktiles per exp group

    with (
        tc.tile_pool(name="singles", bufs=1) as singles,
        tc.tile_pool(name="work", bufs=3) as work,
        tc.tile_pool(name="psum", bufs=2, space="PSUM") as psum,
    ):
        xT_src = xT_d.rearrange("(t p) s -> p t s", p=P)

        # ---- startup: only what Q-proj pair 0 needs, first ----
        w_bf = {}
        w_bf["wq"] = singles.tile([P, DT, D], BF16, name="wq_bf")
        nc.sync.dma_start(w_bf["wq"], w_dram["wq"].rearrange(
            "(t p) n -> p t n", p=P))
        b_col = {}
        b_col["bq"] = singles.tile([P, DT], F32, name="bq_col")
        nc.sync.dma_start(b_col["bq"], b_dram["bq"].rearrange(
            "(t p) -> p t", p=P))
        xTq = singles.tile([P, DT, sq], BF16)
        nc.sync.dma_start(xTq, xqT_d.rearrange("(t p) s -> p t s", p=P))

        QT = singles.tile([P, DT, sq], BF16)

        def qproj(j, nq):
            psQ = psum.tile([P, QC], F32, tag="proj", name="psQ")
            for kt in range(DT):
                nc.tensor.matmul(
                    psQ, w_bf["wq"][:, kt, j * P:(j + 1) * P],
                    xTq[:, kt, nq * QC:(nq + 1) * QC],
                    start=(kt == 0), stop=(kt == DT - 1))
            nc.vector.tensor_scalar(
                QT[:, j, nq * QC:(nq + 1) * QC], psQ,
                b_col["bq"][:, j:j + 1], 0.0, op0=ALU.add, op1=ALU.max)

        qproj(0, 0)

        # ---- rest of the weights / biases ----
        for n in ("wk", "wv", "wo"):
            wb = singles.tile([P, DT, D], BF16, name=f"{n}_bf")
            nc.sync.dma_start(wb, w_dram[n].rearrange("(t p) n -> p t n", p=P))
            w_bf[n] = wb
        b_col["bk"] = singles.tile([P, DT], F32, name="bk_col")
        nc.sync.dma_start(b_col["bk"], b_dram["bk"].rearrange(
            "(t p) -> p t", p=P))
        b_row = {}
        for n in ("bv", "bo"):
            br = singles.tile([1, D], BF16, name=f"{n}_row")
            nc.sync.dma_start(br, b_dram[n][None, :])
            b_row[n] = br

        # ---- persistent SBUF tensors ----
        xT = singles.tile([P, DT, sk], BF16)
        xT1 = singles.tile([1, sk], BF16)
        nc.vector.memset(xT1, 1.0)
        KT = singles.tile([P, DT, sk], BF16)
        V_pad = singles.tile([P, SK_T, H, DH + 1], BF16)
        nc.vector.memset(V_pad[:, :, :, DH:DH + 1], 1.0)
        OT = singles.tile([P, DT, sq], BF16)
        OT1 = singles.tile([1, sq], BF16)
        nc.vector.memset(OT1, 1.0)

        # PSUM tags: "proj" 2x1 banks, "scores" 1x4 banks, "psU" 2x1 = 8
        def vproj(st):
            psV = psum.tile([P, D], F32, tag="proj", name="psV")
            for kt in range(DT):
                nc.tensor.matmul(
                    psV, xT[:, kt, st * P:(st + 1) * P], w_bf["wv"][:, kt, :],
                    start=(kt == 0), stop=False)
            nc.tensor.matmul(psV, xT1[:, st * P:(st + 1) * P],
                             b_row["bv"], start=False, stop=True)
            nc.vector.tensor_scalar_max(
                V_pad[:, st, :, 0:DH],
                psV.rearrange("p (h d) -> p h d", h=H), 0.0)

        def kproj(j, n):
            psK = psum.tile([P, CH * P], F32, tag="proj", name="psK")
            for kt in range(DT):
                nc.tensor.matmul(
                    psK, w_bf["wk"][:, kt, j * P:(j + 1) * P],
                    xT[:, kt, n * CH * P:(n + 1) * CH * P],
                    start=(kt == 0), stop=(kt == DT - 1))
            nc.vector.tensor_scalar(
                KT[:, j, n * CH * P:(n + 1) * CH * P], psK,
                b_col["bk"][:, j:j + 1], 0.0, op0=ALU.add, op1=ALU.max)

        def attn_group(j, qc, kt0, nkt, psU_A, psU_B):
            """One exp group: nkt ktiles x 2 heads -> one ACT exp op."""
            q0 = qc * QC
            psS = psum.tile([P, 2 * KG * QC], F32, tag="scores", bufs=2,
                            name="psS")
            for i in range(nkt):
                kt = kt0 + i
                nc.tensor.matmul(
                    psS[:, (2 * i) * QC:(2 * i + 1) * QC],
                    KT[0:DH, j, kt * P:(kt + 1) * P],
                    QT[0:DH, j, q0:q0 + QC], start=True, stop=True)
                nc.tensor.matmul(
                    psS[:, (2 * i + 1) * QC:(2 * i + 2) * QC],
                    KT[DH:P, j, kt * P:(kt + 1) * P],
                    QT[DH:P, j, q0:q0 + QC], start=True, stop=True)
            pT = work.tile([P, 2 * KG * QC], BF16, tag="pT", name="pT")
            nc.scalar.activation(pT[:, :2 * nkt * QC], psS[:, :2 * nkt * QC],
                                 AF.Exp, scale=0.125)
            for i in range(nkt):
                kt = kt0 + i
                first, last = (kt == 0), (kt == SK_T - 1)
                nc.tensor.matmul(psU_A, V_pad[:, kt, 2 * j, :],
                                 pT[:, (2 * i) * QC:(2 * i + 1) * QC],
                                 start=first, stop=last)
                nc.tensor.matmul(psU_B, V_pad[:, kt, 2 * j + 1, :],
                                 pT[:, (2 * i + 1) * QC:(2 * i + 2) * QC],
                                 start=first, stop=last)

        def attn_finish(j, qc, psU_A, psU_B):
            """Copy U out of PSUM fast (frees both accumulators), then
            normalize off the critical path."""
            q0 = qc * QC
            ucs = []
            for psU in (psU_A, psU_B):
                uc = work.tile([DH + 1, QC], F32, tag="ucopy", bufs=4,
                               name="uc")
                nc.vector.tensor_copy(uc, psU)
                ucs.append(uc)
            for uc, h0 in zip(ucs, (0, DH)):
                recip = work.tile([1, QC], F32, tag="recip", bufs=4,
                                  name="recip")
                nc.vector.reciprocal(recip, uc[DH:DH + 1, :])
                brc = work.tile([DH, QC], F32, tag="brc", bufs=4, name="brc")
                nc.gpsimd.partition_broadcast(brc, recip)
                nc.vector.tensor_mul(
                    OT[h0:h0 + DH, j, q0:q0 + QC], uc[0:DH, :], brc)

        def attn_span(j, qc, kts, psU):
            for kt0 in range(kts[0], kts[0] + len(kts), KG):
                nkt = min(KG, kts[-1] + 1 - kt0)
                attn_group(j, qc, kt0, nkt, psU[0], psU[1])
            if kts[-1] == SK_T - 1:
                attn_finish(j, qc, psU[0], psU[1])

        def new_psU():
            a = psum.tile([DH + 1, QC], F32, tag="psU", name="psU_A")
            b = psum.tile([DH + 1, QC], F32, tag="psU", name="psU_B")
            return (a, b)

        # ---- chunk loop: x load + V proj + K proj(pair 0) + attn(0, 0) ----
        psU0 = new_psU()
        for n in range(NCH):
            s0, s1 = n * CH * P, (n + 1) * CH * P
            nc.sync.dma_start(xT[:, :, s0:s1], xT_src[:, :, s0:s1])
            for st in range(n * CH, (n + 1) * CH):
                vproj(st)
            kproj(0, n)
            attn_span(0, 0, list(range(n * CH, (n + 1) * CH)), psU0)

        # ---- remaining attention; projections hidden between blocks ----
        for qc in range(1, NQC):
            qproj(0, qc)
            psU01 = new_psU()
            attn_span(0, qc, list(range(SK_T)), psU01)
        for j in range(1, DT):
            for n in range(NCH):
                kproj(j, n)
            for qc in range(NQC):
                qproj(j, qc)
                psU = new_psU()
                attn_span(j, qc, list(range(SK_T)), psU)

        # ---- output projection ----
        for qt in range(SQ_T):
            psO = psum.tile([P, D], F32, tag="proj", name="psO")
            for j in range(DT):
                nc.tensor.matmul(psO, OT[:, j, qt * P:(qt + 1) * P],
                                 w_bf["wo"][:, j, :],
                                 start=(j == 0), stop=False)
            nc.tensor.matmul(psO, OT1[:, qt * P:(qt + 1) * P],
                             b_row["bo"], start=False, stop=True)
            o_sb = work.tile([P, D], F32, tag="osb", bufs=2, name="o_sb")
            nc.scalar.activation(o_sb, psO, AF.Relu)
            nc.sync.dma_start(out[qt * P:(qt + 1) * P, :], o_sb)


_NC_CACHE = {}


def _get_nc(sk=S, sq=SQ_FULL):
    key = (sk, sq)
    if key not in _NC_CACHE:
        _NC_CACHE[key] = build_mha(sk, sq)
    return _NC_CACHE[key]


def kernel(x, Wq, bq, Wk, bk, Wv, bv, Wo, bo, **run_kwargs):
    """Full-input entry point: shards across 8 NeuronCores, returns full out."""
    bf = ml_dtypes.bfloat16
    x = np.asarray(x, dtype=np.float32)
    w_bf = {
        "wq": np.ascontiguousarray(np.asarray(Wq, np.float32).astype(bf)),
        "wk": np.ascontiguousarray(np.asarray(Wk, np.float32).astype(bf)),
        "wv": np.ascontiguousarray(np.asarray(Wv, np.float32).astype(bf)),
        "wo": np.ascontiguousarray(np.asarray(Wo, np.float32).astype(bf)),
    }
    biases = {
        "bq": np.ascontiguousarray(np.asarray(bq, np.float32)),
        "bk": np.ascontiguousarray(np.asarray(bk, np.float32)),
        "bv": np.ascontiguousarray(np.asarray(bv, np.float32).astype(bf)),
        "bo": np.ascontiguousarray(np.asarray(bo, np.float32).astype(bf)),
    }
    # host-side layout prep: bf16 cast + feature-major transpose per batch
    xT_b = [np.ascontiguousarray(x[b].T.astype(bf)) for b in range(B)]

    nc = _get_nc()
    in_maps = []
    for c in range(NCORES):
        b, qo = divmod(c, QSPLIT)
        m = dict(w_bf)
        m.update(biases)
        m["xT_bf"] = xT_b[b]
        m["xqT_bf"] = np.ascontiguousarray(
            xT_b[b][:, qo * SQ_FULL:(qo + 1) * SQ_FULL])
        in_maps.append(m)
    res = bass_utils.run_bass_kernel_spmd(
        nc, in_maps, core_ids=list(range(NCORES)), **run_kwargs)
    full = np.empty((B, S, D), np.float32)
    for c in range(NCORES):
        b, qo = divmod(c, QSPLIT)
        full[b, qo * SQ_FULL:(qo + 1) * SQ_FULL] = res.results[c]["out"]
    if run_kwargs:
        return full, res
    return full
